# revision 21
# baseline (speedup 1.0000x reference)
"""Trainium2 Bass kernel for nn_AttentionStem (5x5 local attention stem, stride 2).

Self-contained: hardcodes shapes B=8, CIN=64, H=W=128, OUT_CH=128, M=2, K=5.
Data-parallel over batch: one batch element per NeuronCore (8 cores).

Math (per batch):
  scores[k,(h,w)] = x_s(2h,2w)^T G x(p'_k),  G = w_q^T w_k   (q/k folded)
  a_m[key,pos]    = exp(score) * wpos_m(dh,dw) * band
  out[pos,ch]     = sum_m wv_m^T ( sum_d xTe_r(d)^T a_m ) / den   (V folded
                    through the apply: Q_m[c,pos] = sum_keys a_m x[c,key],
                    then out = wv_m^T Q_m -- the big V tensor is never
                    materialized, killing the PSUM->SBUF V copy.)

v2 layout:
  - x bf16, even image rows on partitions 0:64, odd on 64:128 (ST row-tiling).
  - xTe: x transposed to [imgcol, row, ch] + ones channel + 2 pad rows each
    side (host-prepared). stage-A lhsT; ones channel accumulates den in Q
    row 64; pad rows make OOB key rows contribute sum(wpos) to den only.
  - per-slab chain: ST mms -> ACT exp -> DVE mask -> (3 iters later)
    stage-A mms -> ACT Q-copy -> stage-B mms -> DVE den/recip/scale -> DMA.
  - out stored bf16, host casts to f32.
"""

import sys

for _p in ("/opt/pypackages", "/opt/trn_rl_repo"):
    if _p not in sys.path:
        sys.path.insert(0, _p)

from contextlib import ExitStack

import ml_dtypes
import numpy as np

import concourse.bacc as bacc
import concourse.bass as bass
import concourse.mybir as mybir
from concourse.bass_utils import run_bass_kernel_spmd
from concourse.tile import TileContext

F32 = mybir.dt.float32
BF16 = mybir.dt.bfloat16

NCORES = 8
CIN = 64
IMG = 128          # input H = W
OC = 128           # out channels
HO = 64            # output H = W
NPAIR = 32         # output row pairs
SLABW = 896        # trimmed transposed-score slab width
XTW = 65           # xTe row width: 64 channels + ones
XTROWS = 132       # 2 pad + 128 + 2 pad rows in xTe

# d (= key row r - 4j for pair j) -> col offset of its 128-col block in a slab.
# Blocks from even key rows (PE row-tile 0) sit in PSUM bank A (cols 0:512),
# odd-row blocks (tile 1) in bank B (cols 512:896) -- concurrent row-tiled
# matmuls writing the same PSUM bank hang TRN2.
OFF_OF_D = {4: 0, 0: 128, 2: 256, -2: 384, 1: 512, 3: 640, -1: 768}

DELAY_A = 4        # stage-A for pair s-DELAY_A is emitted in iteration s
                   # (2-iter slack so DVE finish-bursts never stall the PE FIFO)


def make_wpos(row_emb, col_emb, mix_emb):
    a = mix_emb.T.astype(np.float64) @ row_emb.astype(np.float64)  # [2,5]
    b = mix_emb.T.astype(np.float64) @ col_emb.astype(np.float64)  # [2,5]
    wp = a[:, :, None] + b[:, None, :]                             # [2,5,5]
    wp = wp - wp.max(axis=0, keepdims=True)
    e = np.exp(wp)
    wp = e / e.sum(axis=0, keepdims=True)
    return wp.reshape(2, 25).astype(np.float32)                    # [m, dh*5+dw]


def make_masks(wpos):
    """wpos-weighted band masks in the trimmed ST layout.

    Returns [128 (kcol), 2 (m), 896] f32; block at OFF_OF_D[d] holds the
    masks for key row r = 4j + d of pair j, cols rho*64 + w."""
    wm = np.zeros((128, 2, SLABW), np.float32)
    for d, base in OFF_OF_D.items():
        for rho in (0, 1):
            dh = d + 2 - 2 * rho
            if not 0 <= dh < 5:
                continue
            for w in range(64):
                for dw in range(5):
                    kc = 2 * w + dw - 2
                    if 0 <= kc < 128:
                        wm[kc, :, base + rho * 64 + w] = wpos[:, dh * 5 + dw]
    return wm


def make_oob():
    """#window entries with out-of-image column, per position in a pair."""
    oob = np.zeros((128, 1), np.float32)
    for rho in (0, 1):
        for w in range(64):
            cnt = sum(1 for dw in range(5) if not 0 <= 2 * w + dw - 2 < 128)
            oob[rho * 64 + w, 0] = 5.0 * cnt
    return oob


def _ap(t, off, dims, p0=0, pn=None):
    a = t[:]
    np_ = pn if pn is not None else a.ap[0][1]
    return bass.AP(tensor=a.tensor, offset=off + p0 * a.ap[0][0],
                   ap=[[a.ap[0][0], np_]] + [list(d) for d in dims])


def build_nc():
    nc = bacc.Bacc("TRN2", target_bir_lowering=False, debug=False, num_devices=NCORES)

    xe_d = nc.dram_tensor("xe", [CIN, 64, IMG], BF16, kind="ExternalInput")
    xo_d = nc.dram_tensor("xo", [CIN, 64, IMG], BF16, kind="ExternalInput")
    xte_d = nc.dram_tensor("xte", [128, XTROWS * XTW], BF16, kind="ExternalInput")
    g2_d = nc.dram_tensor("g2", [CIN, 128], BF16, kind="ExternalInput")
    wv_d = nc.dram_tensor("wve", [XTW, 258], BF16, kind="ExternalInput")
    # wmask + a trailing 128-col block whose kc=0 row holds the col-OOB count
    # (stage-A adds it to the den row via the pad-ones lhsT).
    wm_d = nc.dram_tensor("wmask", [128, 2 * SLABW + 128], BF16, kind="ExternalInput")
    out_d = nc.dram_tensor("out", [HO * HO, OC], BF16, kind="ExternalOutput")

    EXP = mybir.ActivationFunctionType.Exp

    with TileContext(nc) as tc, ExitStack() as ctx:
        sg = ctx.enter_context(tc.tile_pool(name="singles", bufs=1))
        # x: partitions 0:64 even image rows, 64:128 odd rows; 64 rows x 128 cols
        x_sb = sg.tile([128, 64 * IMG], BF16)
        xte_sb = sg.tile([128, XTROWS * XTW], BF16)
        y_sb = sg.tile([128, 4096], BF16)            # queries, dup on both halves
        wm_sb = sg.tile([128, 2 * SLABW + 128], BF16)
        g2_sb = sg.tile([64, 128], BF16)
        wv_sb = sg.tile([XTW, 258], BF16)
        scr_sb = sg.tile([64, 512], BF16)            # HAM warmup scratch

        # sync queue: small critical constants first
        nc.sync.dma_start(out=g2_sb[:], in_=g2_d.ap())
        nc.sync.dma_start(out=wm_sb[:], in_=wm_d.ap())
        nc.sync.dma_start(out=wv_sb[:], in_=wv_d.ap())
        nc.gpsimd.memset(scr_sb[:], 0.0)

        # gpsimd ring (cheap 25ns issue): x chunks only, E/O interleaved.
        # E/O chunk k: 8 packed rows (img rows 16k..16k+15), needed by slab 4k.
        # xte chunks ride the sync ring after the constants (needed later:
        # stage-A runs DELAY_A iterations behind).
        def x_chunk(c8):
            dst_e = _ap(x_sb, c8 * 8 * IMG, [[1, 8 * IMG]], 0, 64)
            dst_o = _ap(x_sb, c8 * 8 * IMG, [[1, 8 * IMG]], 64, 64)
            nc.gpsimd.dma_start(out=dst_e, in_=xe_d.ap()[:, c8 * 8:(c8 + 1) * 8, :])
            nc.gpsimd.dma_start(out=dst_o, in_=xo_d.ap()[:, c8 * 8:(c8 + 1) * 8, :])

        def xte_chunk(c):
            r0 = c * 22
            dst = _ap(xte_sb, r0 * XTW, [[1, 22 * XTW]])
            nc.sync.dma_start(out=dst, in_=xte_d.ap()[:, r0 * XTW:(r0 + 22) * XTW])

        for k in range(8):
            x_chunk(k)
        for c in range(6):
            xte_chunk(c)

        def xrow(r):
            # key row r: [64 partitions (channels), 128 cols] on its parity half
            p = (r & 1) * 64
            return x_sb[p:p + 64, (r >> 1) * IMG:(r >> 1) * IMG + IMG]

        with tc.tile_pool(name="stp", bufs=2, space="PSUM") as stpool, \
             tc.tile_pool(name="ybuf", bufs=1, space="PSUM") as ybuf, \
             tc.tile_pool(name="qp", bufs=1, space="PSUM") as qpool, \
             tc.tile_pool(name="apb", bufs=2, space="PSUM") as apool, \
             tc.tile_pool(name="e2t", bufs=3) as e2t, \
             tc.tile_pool(name="a0p", bufs=7) as a0p, \
             tc.tile_pool(name="qsb", bufs=3) as qsb, \
             tc.tile_pool(name="outsb", bufs=3) as outsb, \
             tc.tile_pool(name="dens", bufs=4) as dens:

            A = {}       # slab -> a01 tile [128, 2*SLABW]
            QS = {}      # pair -> qs tile [65, 256]
            AP2 = {}     # pair-group t -> apool tile [128, 258]

            # HAM warmup: ~3.4us of dummy matmuls (no DMA deps) so the PE
            # clock is at 8/8 when the first real matmul issues.
            wup = stpool.tile([128, 1024], F32, name="wup", tag="st")
            for w in range(8):
                nc.tensor.matmul(wup[:, (w % 2) * 512:(w % 2) * 512 + 512],
                                 scr_sb[0:64, 0:128], scr_sb[0:64, 0:512],
                                 start=True, stop=True, skip_group_check=True)

            def make_y(k):
                # y chunk k: queries for strided rows 8k..8k+8 -> y_sb cols
                # k*512..(k+1)*512; needs x even chunk k only.
                yp = ybuf.tile([128, 512], F32)
                rhs = _ap(x_sb, k * 8 * IMG, [[IMG, 8], [2, 64]], 0, 64)
                nc.tensor.matmul(yp[:], g2_sb[:], rhs, start=True, stop=True)
                dst = y_sb[:, k * 512:(k + 1) * 512]
                if k % 2 == 0:
                    nc.scalar.copy(dst, yp[:])
                else:
                    nc.vector.tensor_copy(dst, yp[:])

            def make_slab(s):
                stp = stpool.tile([128, 1024], F32, tag="st")
                n = min(2, NPAIR - s) * 128
                # t=0 (tile0, bank A): pairs s-1, s -> cols 0:256
                if s == 0:
                    nc.tensor.matmul(stp[:, 128:256], xrow(0),
                                     y_sb[0:64, 0:128], start=True, stop=True)
                else:
                    nc.tensor.matmul(stp[:, 0:256], xrow(4 * s),
                                     y_sb[0:64, (s - 1) * 128:(s + 1) * 128],
                                     start=True, stop=True)
                # t=1 (tile1, bank B): pair s -> cols 512:640
                nc.tensor.matmul(stp[:, 512:640], xrow(4 * s + 1),
                                 y_sb[64:128, s * 128:(s + 1) * 128],
                                 start=True, stop=True)
                # t=2 (tile0, bank A): pairs s, s+1 -> cols 256:256+n
                nc.tensor.matmul(stp[:, 256:256 + n], xrow(4 * s + 2),
                                 y_sb[0:64, s * 128:s * 128 + n],
                                 start=True, stop=True)
                # t=3 (tile1, bank B): pairs s, s+1 -> cols 640:640+n
                nc.tensor.matmul(stp[:, 640:640 + n], xrow(4 * s + 3),
                                 y_sb[64:128, s * 128:s * 128 + n],
                                 start=True, stop=True)
                e2 = e2t.tile([128, SLABW], BF16)
                nc.scalar.activation(out=e2[:], in_=stp[:, 0:SLABW], func=EXP)
                a01 = a0p.tile([128, 2 * SLABW], BF16)
                e2r = bass.AP(tensor=e2[:].tensor, offset=e2[:].offset,
                              ap=[list(e2[:].ap[0]), [0, 2], [1, SLABW]])
                nc.vector.tensor_mul(a01[:], e2r, wm_sb[:, 0:2 * SLABW])
                A[s] = a01
                A.pop(s - (DELAY_A + 2), None)

            def stage_a(j):
                # Q_m[c,pos] = sum_d sum_kc xTe_r[kc,c] a_m[kc,pos]; Q row 64
                # accumulates den (ones channel / wm pad rows). m=0 also adds
                # the col-OOB count block (pad-ones lhsT x oob block).
                qp = qpool.tile([65, 256], F32)
                pad0 = _ap(xte_sb, 0, [[1, XTW]])
                for m in (0, 1):
                    ds = (0, 1, 2, -2, -1, 3, 4)
                    if m == 0:
                        nc.tensor.matmul(qp[:, 0:128], pad0,
                                         wm_sb[:, 2 * SLABW:2 * SLABW + 128],
                                         start=True, stop=False,
                                         skip_group_check=True)
                    for idx, d in enumerate(ds):
                        r = 4 * j + d
                        lhsT = _ap(xte_sb, (r + 2) * XTW, [[1, XTW]])
                        if r < 0 or r >= IMG:
                            rhs = wm_sb
                        else:
                            rhs = A[r // 4]
                        off = OFF_OF_D[d] + m * SLABW
                        nc.tensor.matmul(qp[:, m * 128:(m + 1) * 128],
                                         lhsT, rhs[:, off:off + 128],
                                         start=(idx == 0 and m == 1),
                                         stop=idx == len(ds) - 1,
                                         skip_group_check=True)
                qs = qsb.tile([65, 256], BF16)
                nc.scalar.copy(qs[:], qp[:])
                QS[j] = qs

            def stage_b(j):
                # out[pos,ch'] (+den in col 128) = sum_m wv_m^T Qs_m
                t = j // 2
                if j % 2 == 0:
                    AP2[t] = apool.tile([128, 258], F32, name="ap2")
                ap2 = AP2[t]
                qs = QS.pop(j)
                c0 = (j % 2) * 129
                for m in (0, 1):
                    nc.tensor.matmul(ap2[:, c0:c0 + 129],
                                     qs[0:XTW, m * 128:(m + 1) * 128],
                                     wv_sb[:, m * 129:(m + 1) * 129],
                                     start=m == 0, stop=m == 1,
                                     skip_group_check=True)

            def finish_group(t, only_p=None):
                # pairs 2t, 2t+1: rec = 1/den (den already includes oob via
                # stage-A), out = ap[:,0:128] * rec -> bf16 -> DMA per pair.
                last = only_p is not None
                if last and only_p == 0:
                    ap2 = AP2[t]
                else:
                    ap2 = AP2.pop(t)
                ps = (only_p,) if last else (0, 1)
                rec = dens.tile([128, 2], F32)
                nc.vector.reciprocal(rec[:, ps[0]:ps[-1] + 1],
                                     _ap(ap2, 128 + ps[0] * 129,
                                         [[129, len(ps)]]))
                o_sb = outsb.tile([128, 256], BF16)
                rec_b = bass.AP(tensor=rec[:].tensor,
                                offset=rec[:].offset + ps[0],
                                ap=[list(rec[:].ap[0]), [1, len(ps)], [0, 128]])
                src = bass.AP(tensor=ap2[:].tensor,
                              offset=ap2[:].offset + ps[0] * 129,
                              ap=[list(ap2[:].ap[0]), [129, len(ps)], [1, 128]])
                nc.vector.tensor_mul(
                    o_sb[:, ps[0] * 128:(ps[-1] + 1) * 128], src, rec_b)
                for p in ps:
                    j = 2 * t + p
                    q = nc.gpsimd if last else nc.sync
                    q.dma_start(out=out_d.ap()[j * 128:(j + 1) * 128, :],
                                in_=o_sb[:, p * 128:(p + 1) * 128])

            # software-pipelined main loop
            make_y(0)
            make_y(1)
            for s in range(NPAIR):
                make_slab(s)
                # y(k) feeds slabs >= 4k-1; emit at iter 4k-4 (x chunk E_k
                # lands ~1.8k us after DMA start, well before then)
                if s >= 4 and s % 4 == 0 and s // 4 + 1 < 8:
                    make_y(s // 4 + 1)
                if s >= DELAY_A:
                    stage_a(s - DELAY_A)
                if s >= DELAY_A + 1:
                    stage_b(s - DELAY_A - 1)
                if s >= DELAY_A + 2 and (s - DELAY_A) % 2 == 0:
                    finish_group((s - DELAY_A) // 2 - 1)
            for j in range(NPAIR - DELAY_A, NPAIR):
                stage_a(j)
                stage_b(j - 1)
                if j % 2 == 0:
                    finish_group(j // 2 - 1)
            finish_group(NPAIR // 2 - 1, only_p=0)
            stage_b(NPAIR - 1)
            finish_group(NPAIR // 2 - 1, only_p=1)

    nc.compile()
    return nc


_NC_CACHE = None


def kernel(x, w_q, w_k, w_v, row_emb, col_emb, mix_emb):
    global _NC_CACHE
    x = np.asarray(x, np.float32)
    w_q = np.asarray(w_q, np.float32)
    w_k = np.asarray(w_k, np.float32)
    w_v = np.asarray(w_v, np.float32)
    row_emb = np.asarray(row_emb, np.float32)
    col_emb = np.asarray(col_emb, np.float32)
    mix_emb = np.asarray(mix_emb, np.float32)

    G = (w_q.T @ w_k).astype(ml_dtypes.bfloat16)          # [64, 64]
    g2 = np.hstack([G, G])                                # [64, 128]
    wpos = make_wpos(row_emb, col_emb, mix_emb)
    wmask = np.zeros((128, 2 * SLABW + 128), np.float32)
    wmask[:, 0:2 * SLABW] = make_masks(wpos).reshape(128, 2 * SLABW)
    wmask[0, 2 * SLABW:] = make_oob()[:, 0]
    wmask = wmask.astype(ml_dtypes.bfloat16)

    # wv_ext: [65, 258]; block m: rows 0:64 col j = w_v[2j+m, c]; row 64 is
    # the den pass-through (-> out col 128).
    wve = np.zeros((XTW, 258), np.float32)
    for m in (0, 1):
        wve[0:64, m * 129:m * 129 + 128] = w_v[m::2, :].T
        wve[64, m * 129 + 128] = 1.0
    wve = wve.astype(ml_dtypes.bfloat16)

    xb = x.astype(ml_dtypes.bfloat16)                      # [B, 64, 128, 128]
    xe = np.ascontiguousarray(xb[:, :, 0::2, :])           # [B, 64, 64, 128]
    xo = np.ascontiguousarray(xb[:, :, 1::2, :])

    # xTe: [B, imgcol 128, row 132, ch 65]; ones channel 64; pad rows 0/1 and
    # 130/131 are zero except the ones channel.
    xte = np.zeros((NCORES, 128, XTROWS, XTW), np.float32)
    xte[:, :, 2:130, 0:64] = x.transpose(0, 3, 2, 1)
    xte[:, :, :, 64] = 1.0
    xte = xte.reshape(NCORES, 128, XTROWS * XTW).astype(ml_dtypes.bfloat16)

    if _NC_CACHE is None:
        _NC_CACHE = build_nc()
    nc = _NC_CACHE

    in_maps = []
    for b in range(NCORES):
        in_maps.append({
            "xe": xe[b],
            "xo": xo[b],
            "xte": xte[b],
            "g2": g2,
            "wve": wve,
            "wmask": wmask,
        })
    res = run_bass_kernel_spmd(nc, in_maps, core_ids=list(range(NCORES)))
    out = np.stack([res.results[b]["out"].astype(np.float32).T.reshape(OC, HO, HO)
                    for b in range(NCORES)])
    return out


# revision 25
# speedup vs baseline: 1.0895x; 1.0895x over previous
"""Trainium2 Bass kernel for nn_AttentionStem (5x5 local attention stem, stride 2).

Self-contained: hardcodes shapes B=8, CIN=64, H=W=128, OUT_CH=128, M=2, K=5.
Data-parallel over batch: one batch element per NeuronCore (8 cores).

Math (per batch):
  scores[k,(h,w)] = x_s(2h,2w)^T G x(p'_k),  G = w_q^T w_k   (q/k folded)
  a_m[key,pos]    = exp(score) * wpos_m(dh,dw) * band
  out[pos,ch]     = sum_m wv_m^T ( sum_d xTe_r(d)^T a_m ) / den   (V folded
                    through the apply: Q_m[c,pos] = sum_keys a_m x[c,key],
                    then out = wv_m^T Q_m -- the big V tensor is never
                    materialized, killing the PSUM->SBUF V copy.)

v2 layout:
  - x bf16, even image rows on partitions 0:64, odd on 64:128 (ST row-tiling).
  - xTe: x transposed to [imgcol, row, ch] + ones channel + 2 pad rows each
    side (host-prepared). stage-A lhsT; ones channel accumulates den in Q
    row 64; pad rows make OOB key rows contribute sum(wpos) to den only.
  - per-slab chain: ST mms -> ACT exp -> DVE mask -> (3 iters later)
    stage-A mms -> ACT Q-copy -> stage-B mms -> DVE den/recip/scale -> DMA.
  - out stored bf16, host casts to f32.
"""

import sys

for _p in ("/opt/pypackages", "/opt/trn_rl_repo"):
    if _p not in sys.path:
        sys.path.insert(0, _p)

from contextlib import ExitStack

import ml_dtypes
import numpy as np

import concourse.bacc as bacc
import concourse.bass as bass
import concourse.mybir as mybir
from concourse.bass_utils import run_bass_kernel_spmd
from concourse.tile import TileContext

F32 = mybir.dt.float32
BF16 = mybir.dt.bfloat16

NCORES = 8
CIN = 64
IMG = 128          # input H = W
OC = 128           # out channels
HO = 64            # output H = W
NPAIR = 32         # output row pairs
SLABW = 896        # trimmed transposed-score slab width
XTW = 65           # xTe row width: 64 channels + ones
XTROWS = 132       # 2 pad + 128 + 2 pad rows in xTe

# d (= key row r - 4j for pair j) -> col offset of its 128-col block in a slab.
# Blocks from even key rows (PE row-tile 0) sit in PSUM bank A (cols 0:512),
# odd-row blocks (tile 1) in bank B (cols 512:896) -- concurrent row-tiled
# matmuls writing the same PSUM bank hang TRN2.
OFF_OF_D = {4: 0, 0: 128, 2: 256, -2: 384, 1: 512, 3: 640, -1: 768}

DELAY_A = 4        # stage-A for pair s-DELAY_A is emitted in iteration s
                   # (2-iter slack so DVE finish-bursts never stall the PE FIFO)


def make_wpos(row_emb, col_emb, mix_emb):
    a = mix_emb.T.astype(np.float64) @ row_emb.astype(np.float64)  # [2,5]
    b = mix_emb.T.astype(np.float64) @ col_emb.astype(np.float64)  # [2,5]
    wp = a[:, :, None] + b[:, None, :]                             # [2,5,5]
    wp = wp - wp.max(axis=0, keepdims=True)
    e = np.exp(wp)
    wp = e / e.sum(axis=0, keepdims=True)
    return wp.reshape(2, 25).astype(np.float32)                    # [m, dh*5+dw]


def make_masks(wpos):
    """wpos-weighted band masks in the trimmed ST layout.

    Returns [128 (kcol), 2 (m), 896] f32; block at OFF_OF_D[d] holds the
    masks for key row r = 4j + d of pair j, cols rho*64 + w."""
    wm = np.zeros((128, 2, SLABW), np.float32)
    for d, base in OFF_OF_D.items():
        for rho in (0, 1):
            dh = d + 2 - 2 * rho
            if not 0 <= dh < 5:
                continue
            for w in range(64):
                for dw in range(5):
                    kc = 2 * w + dw - 2
                    if 0 <= kc < 128:
                        wm[kc, :, base + rho * 64 + w] = wpos[:, dh * 5 + dw]
    return wm


def make_oob():
    """#window entries with out-of-image column, per position in a pair."""
    oob = np.zeros((128, 1), np.float32)
    for rho in (0, 1):
        for w in range(64):
            cnt = sum(1 for dw in range(5) if not 0 <= 2 * w + dw - 2 < 128)
            oob[rho * 64 + w, 0] = 5.0 * cnt
    return oob


def _ap(t, off, dims, p0=0, pn=None):
    a = t[:]
    np_ = pn if pn is not None else a.ap[0][1]
    return bass.AP(tensor=a.tensor, offset=off + p0 * a.ap[0][0],
                   ap=[[a.ap[0][0], np_]] + [list(d) for d in dims])


def build_nc():
    nc = bacc.Bacc("TRN2", target_bir_lowering=False, debug=False, num_devices=NCORES)

    xe_d = nc.dram_tensor("xe", [CIN, 64, IMG], BF16, kind="ExternalInput")
    xo_d = nc.dram_tensor("xo", [CIN, 64, IMG], BF16, kind="ExternalInput")
    xte_d = nc.dram_tensor("xte", [128, XTROWS * XTW], BF16, kind="ExternalInput")
    g2_d = nc.dram_tensor("g2", [CIN, 128], BF16, kind="ExternalInput")
    wv_d = nc.dram_tensor("wve", [XTW, 258], BF16, kind="ExternalInput")
    # wmask + a trailing 2x128-col block whose kc=0 row holds the col-OOB
    # count twice (stage-A adds it to both pairs' den rows in one matmul).
    wm_d = nc.dram_tensor("wmask", [128, 2 * SLABW + 256], BF16, kind="ExternalInput")
    out_d = nc.dram_tensor("out", [HO * HO, OC], BF16, kind="ExternalOutput")

    EXP = mybir.ActivationFunctionType.Exp

    with TileContext(nc) as tc, ExitStack() as ctx:
        sg = ctx.enter_context(tc.tile_pool(name="singles", bufs=1))
        # x: partitions 0:64 even image rows, 64:128 odd rows; 64 rows x 128 cols
        x_sb = sg.tile([128, 64 * IMG], BF16)
        xte_sb = sg.tile([128, XTROWS * XTW], BF16)
        y_sb = sg.tile([128, 4096], BF16)            # queries, dup on both halves
        wm_sb = sg.tile([128, 2 * SLABW + 256], BF16)
        g2_sb = sg.tile([64, 128], BF16)
        wv_sb = sg.tile([XTW, 258], BF16)
        scr_sb = sg.tile([64, 512], BF16)            # HAM warmup scratch

        # sync queue: small critical constants first
        nc.sync.dma_start(out=g2_sb[:], in_=g2_d.ap())
        nc.sync.dma_start(out=wm_sb[:], in_=wm_d.ap())
        nc.sync.dma_start(out=wv_sb[:], in_=wv_d.ap())
        nc.gpsimd.memset(scr_sb[:], 0.0)

        # gpsimd ring (cheap 25ns issue): x chunks only, E/O interleaved.
        # E/O chunk k: 8 packed rows (img rows 16k..16k+15), needed by slab 4k.
        # xte chunks ride the sync ring after the constants (needed later:
        # stage-A runs DELAY_A iterations behind).
        def x_chunk(c8):
            dst_e = _ap(x_sb, c8 * 8 * IMG, [[1, 8 * IMG]], 0, 64)
            dst_o = _ap(x_sb, c8 * 8 * IMG, [[1, 8 * IMG]], 64, 64)
            nc.gpsimd.dma_start(out=dst_e, in_=xe_d.ap()[:, c8 * 8:(c8 + 1) * 8, :])
            nc.gpsimd.dma_start(out=dst_o, in_=xo_d.ap()[:, c8 * 8:(c8 + 1) * 8, :])

        def xte_chunk(c):
            r0 = c * 22
            dst = _ap(xte_sb, r0 * XTW, [[1, 22 * XTW]])
            nc.sync.dma_start(out=dst, in_=xte_d.ap()[:, r0 * XTW:(r0 + 22) * XTW])

        for k in range(8):
            x_chunk(k)
        for c in range(6):
            xte_chunk(c)

        def xrow(r):
            # key row r: [64 partitions (channels), 128 cols] on its parity half
            p = (r & 1) * 64
            return x_sb[p:p + 64, (r >> 1) * IMG:(r >> 1) * IMG + IMG]

        with tc.tile_pool(name="stp", bufs=1, space="PSUM") as stpool, \
             tc.tile_pool(name="ybuf", bufs=1, space="PSUM") as ybuf, \
             tc.tile_pool(name="qp", bufs=1, space="PSUM") as qpool, \
             tc.tile_pool(name="apb", bufs=2, space="PSUM") as apool, \
             tc.tile_pool(name="e2t", bufs=2) as e2t, \
             tc.tile_pool(name="a0p", bufs=5) as a0p, \
             tc.tile_pool(name="qsb", bufs=2) as qsb, \
             tc.tile_pool(name="outsb", bufs=3) as outsb, \
             tc.tile_pool(name="dens", bufs=4) as dens:

            A = {}       # slab -> (a01 group tile, col base)
            QS = {}      # group -> qs tile [65, 512]
            AP2 = {}     # group -> apool tile [128, 258]

            # HAM warmup: ~3.4us of dummy matmuls (no DMA deps) so the PE
            # clock is at 8/8 when the first real matmul issues.
            wup = stpool.tile([128, 2048], F32, name="wup", tag="st")
            for w in range(8):
                nc.tensor.matmul(wup[:, (w % 2) * 512:(w % 2) * 512 + 512],
                                 scr_sb[0:64, 0:128], scr_sb[0:64, 0:512],
                                 start=True, stop=True, skip_group_check=True)

            def make_y(k):
                # y chunk k: queries for strided rows 8k..8k+8 -> y_sb cols
                # k*512..(k+1)*512; needs x even chunk k only.
                yp = ybuf.tile([128, 512], F32)
                rhs = _ap(x_sb, k * 8 * IMG, [[IMG, 8], [2, 64]], 0, 64)
                nc.tensor.matmul(yp[:], g2_sb[:], rhs, start=True, stop=True)
                dst = y_sb[:, k * 512:(k + 1) * 512]
                if k % 2 == 0:
                    nc.scalar.copy(dst, yp[:])
                else:
                    nc.vector.tensor_copy(dst, yp[:])

            def st_mms(s, stp, base):
                n = min(2, NPAIR - s) * 128
                # t=0 (tile0, bank A): pairs s-1, s -> cols 0:256
                if s == 0:
                    nc.tensor.matmul(stp[:, base + 128:base + 256], xrow(0),
                                     y_sb[0:64, 0:128], start=True, stop=True)
                else:
                    nc.tensor.matmul(stp[:, base:base + 256], xrow(4 * s),
                                     y_sb[0:64, (s - 1) * 128:(s + 1) * 128],
                                     start=True, stop=True)
                # t=1 (tile1, bank B): pair s -> cols 512:640
                nc.tensor.matmul(stp[:, base + 512:base + 640], xrow(4 * s + 1),
                                 y_sb[64:128, s * 128:(s + 1) * 128],
                                 start=True, stop=True)
                # t=2 (tile0, bank A): pairs s, s+1 -> cols 256:256+n
                nc.tensor.matmul(stp[:, base + 256:base + 256 + n],
                                 xrow(4 * s + 2),
                                 y_sb[0:64, s * 128:s * 128 + n],
                                 start=True, stop=True)
                # t=3 (tile1, bank B): pairs s, s+1 -> cols 640:640+n
                nc.tensor.matmul(stp[:, base + 640:base + 640 + n],
                                 xrow(4 * s + 3),
                                 y_sb[64:128, s * 128:s * 128 + n],
                                 start=True, stop=True)

            def make_group(G):
                # two slabs 2G, 2G+1: 8 ST matmuls, one exp, one mask op
                stp = stpool.tile([128, 2048], F32, tag="st")
                st_mms(2 * G, stp, 0)
                st_mms(2 * G + 1, stp, 1024)
                e2 = e2t.tile([128, 2 * SLABW], BF16)
                src = bass.AP(tensor=stp[:].tensor, offset=stp[:].offset,
                              ap=[list(stp[:].ap[0]), [1024, 2], [1, SLABW]])
                nc.scalar.activation(out=e2[:], in_=src, func=EXP)
                a01 = a0p.tile([128, 4 * SLABW], BF16)
                e2b = bass.AP(tensor=e2[:].tensor, offset=e2[:].offset,
                              ap=[list(e2[:].ap[0]), [SLABW, 2], [0, 2],
                                  [1, SLABW]])
                wmb = bass.AP(tensor=wm_sb[:].tensor, offset=wm_sb[:].offset,
                              ap=[list(wm_sb[:].ap[0]), [0, 2], [SLABW, 2],
                                  [1, SLABW]])
                nc.vector.tensor_mul(a01[:], e2b, wmb)
                A[2 * G] = (a01, 0)
                A[2 * G + 1] = (a01, 2 * SLABW)
                A.pop(2 * G - 8, None)
                A.pop(2 * G - 7, None)

            def stage_a(g):
                # pairs 2g, 2g+1 -> Q group [65, 512]: m0p0|m0p1|m1p0|m1p1.
                # Q row 64 = den (ones channel; pad rows add sum(wpos) for
                # OOB key rows; the oob matmul adds the col-OOB count).
                qgp = qpool.tile([65, 512], F32)
                pad0 = _ap(xte_sb, 0, [[1, XTW]])
                nc.tensor.matmul(qgp[:, 0:256], pad0,
                                 wm_sb[:, 2 * SLABW:2 * SLABW + 256],
                                 start=True, stop=False, skip_group_check=True)
                ds = (0, 1, 2, -2, -1, 3, 4)
                for p in (0, 1):
                    j = 2 * g + p
                    for m in (0, 1):
                        for idx, d in enumerate(ds):
                            r = 4 * j + d
                            lhsT = _ap(xte_sb, (r + 2) * XTW, [[1, XTW]])
                            if r < 0 or r >= IMG:
                                rhs = wm_sb
                                off = OFF_OF_D[d] + m * SLABW
                            else:
                                tile, base = A[r // 4]
                                rhs = tile
                                off = base + OFF_OF_D[d] + m * SLABW
                            nc.tensor.matmul(
                                qgp[:, m * 256 + p * 128:m * 256 + p * 128 + 128],
                                lhsT, rhs[:, off:off + 128],
                                start=(idx == 0 and m == 1),
                                stop=idx == len(ds) - 1,
                                skip_group_check=True)
                qs = qsb.tile([65, 512], BF16)
                nc.scalar.copy(qs[:], qgp[:])
                QS[g] = qs

            def stage_b(g):
                # out[pos,ch'] (+den in col 128) = sum_m wv_m^T Qs_m
                ap2 = apool.tile([128, 258], F32, name="ap2")
                AP2[g] = ap2
                qs = QS.pop(g)
                for p in (0, 1):
                    for m in (0, 1):
                        nc.tensor.matmul(
                            ap2[:, p * 129:p * 129 + 129],
                            qs[0:XTW, m * 256 + p * 128:m * 256 + p * 128 + 128],
                            wv_sb[:, m * 129:(m + 1) * 129],
                            start=m == 0, stop=m == 1,
                            skip_group_check=True)

            def finish_group(g):
                # rec = 1/den (den includes oob via stage-A), out = val*rec
                ap2 = AP2.pop(g)
                rec = dens.tile([128, 2], F32)
                nc.vector.reciprocal(rec[:], _ap(ap2, 128, [[129, 2]]))
                o_sb = outsb.tile([128, 256], BF16)
                rec_b = bass.AP(tensor=rec[:].tensor, offset=rec[:].offset,
                                ap=[list(rec[:].ap[0]), [1, 2], [0, 128]])
                src = bass.AP(tensor=ap2[:].tensor, offset=ap2[:].offset,
                              ap=[list(ap2[:].ap[0]), [129, 2], [1, 128]])
                nc.vector.tensor_mul(o_sb[:], src, rec_b)
                for p in (0, 1):
                    j = 2 * g + p
                    nc.sync.dma_start(out=out_d.ap()[j * 128:(j + 1) * 128, :],
                                      in_=o_sb[:, p * 128:(p + 1) * 128])

            NG = NPAIR // 2
            make_y(0)
            make_y(1)
            for G in range(NG):
                make_group(G)
                if G % 2 == 0 and G // 2 + 2 < 8:
                    make_y(G // 2 + 2)
                if G >= 2:
                    stage_a(G - 2)
                if G >= 3:
                    stage_b(G - 3)
                if G >= 4:
                    finish_group(G - 4)
            for g in range(NG - 2, NG):
                stage_a(g)
                stage_b(g - 1)
                finish_group(g - 2)
            stage_b(NG - 1)
            finish_group(NG - 2)
            finish_group(NG - 1)

    nc.compile()
    return nc


_NC_CACHE = None


def kernel(x, w_q, w_k, w_v, row_emb, col_emb, mix_emb):
    global _NC_CACHE
    x = np.asarray(x, np.float32)
    w_q = np.asarray(w_q, np.float32)
    w_k = np.asarray(w_k, np.float32)
    w_v = np.asarray(w_v, np.float32)
    row_emb = np.asarray(row_emb, np.float32)
    col_emb = np.asarray(col_emb, np.float32)
    mix_emb = np.asarray(mix_emb, np.float32)

    G = (w_q.T @ w_k).astype(ml_dtypes.bfloat16)          # [64, 64]
    g2 = np.hstack([G, G])                                # [64, 128]
    wpos = make_wpos(row_emb, col_emb, mix_emb)
    wmask = np.zeros((128, 2 * SLABW + 256), np.float32)
    wmask[:, 0:2 * SLABW] = make_masks(wpos).reshape(128, 2 * SLABW)
    wmask[0, 2 * SLABW:2 * SLABW + 128] = make_oob()[:, 0]
    wmask[0, 2 * SLABW + 128:] = make_oob()[:, 0]
    wmask = wmask.astype(ml_dtypes.bfloat16)

    # wv_ext: [65, 258]; block m: rows 0:64 col j = w_v[2j+m, c]; row 64 is
    # the den pass-through (-> out col 128).
    wve = np.zeros((XTW, 258), np.float32)
    for m in (0, 1):
        wve[0:64, m * 129:m * 129 + 128] = w_v[m::2, :].T
        wve[64, m * 129 + 128] = 1.0
    wve = wve.astype(ml_dtypes.bfloat16)

    xb = x.astype(ml_dtypes.bfloat16)                      # [B, 64, 128, 128]
    xe = np.ascontiguousarray(xb[:, :, 0::2, :])           # [B, 64, 64, 128]
    xo = np.ascontiguousarray(xb[:, :, 1::2, :])

    # xTe: [B, imgcol 128, row 132, ch 65]; ones channel 64; pad rows 0/1 and
    # 130/131 are zero except the ones channel.
    xte = np.zeros((NCORES, 128, XTROWS, XTW), np.float32)
    xte[:, :, 2:130, 0:64] = x.transpose(0, 3, 2, 1)
    xte[:, :, :, 64] = 1.0
    xte = xte.reshape(NCORES, 128, XTROWS * XTW).astype(ml_dtypes.bfloat16)

    if _NC_CACHE is None:
        _NC_CACHE = build_nc()
    nc = _NC_CACHE

    in_maps = []
    for b in range(NCORES):
        in_maps.append({
            "xe": xe[b],
            "xo": xo[b],
            "xte": xte[b],
            "g2": g2,
            "wve": wve,
            "wmask": wmask,
        })
    res = run_bass_kernel_spmd(nc, in_maps, core_ids=list(range(NCORES)))
    out = np.stack([res.results[b]["out"].astype(np.float32).T.reshape(OC, HO, HO)
                    for b in range(NCORES)])
    return out


# revision 26
# speedup vs baseline: 1.1059x; 1.0151x over previous
"""Trainium2 Bass kernel for nn_AttentionStem (5x5 local attention stem, stride 2).

Self-contained: hardcodes shapes B=8, CIN=64, H=W=128, OUT_CH=128, M=2, K=5.
Data-parallel over batch: one batch element per NeuronCore (8 cores).

Math (per batch):
  scores[k,(h,w)] = x_s(2h,2w)^T G x(p'_k),  G = w_q^T w_k   (q/k folded)
  a_m[key,pos]    = exp(score) * wpos_m(dh,dw) * band
  out[pos,ch]     = sum_m wv_m^T ( sum_d xTe_r(d)^T a_m ) / den   (V folded
                    through the apply: Q_m[c,pos] = sum_keys a_m x[c,key],
                    then out = wv_m^T Q_m -- the big V tensor is never
                    materialized, killing the PSUM->SBUF V copy.)

v2 layout:
  - x bf16, even image rows on partitions 0:64, odd on 64:128 (ST row-tiling).
  - xTe: x transposed to [imgcol, row, ch] + ones channel + 2 pad rows each
    side (host-prepared). stage-A lhsT; ones channel accumulates den in Q
    row 64; pad rows make OOB key rows contribute sum(wpos) to den only.
  - per-slab chain: ST mms -> ACT exp -> DVE mask -> (3 iters later)
    stage-A mms -> ACT Q-copy -> stage-B mms -> DVE den/recip/scale -> DMA.
  - out stored bf16, host casts to f32.
"""

import sys

for _p in ("/opt/pypackages", "/opt/trn_rl_repo"):
    if _p not in sys.path:
        sys.path.insert(0, _p)

from contextlib import ExitStack

import ml_dtypes
import numpy as np

import concourse.bacc as bacc
import concourse.bass as bass
import concourse.mybir as mybir
from concourse.bass_utils import run_bass_kernel_spmd
from concourse.tile import TileContext

F32 = mybir.dt.float32
BF16 = mybir.dt.bfloat16

NCORES = 8
CIN = 64
IMG = 128          # input H = W
OC = 128           # out channels
HO = 64            # output H = W
NPAIR = 32         # output row pairs
SLABW = 896        # trimmed transposed-score slab width
XTW = 65           # xTe row width: 64 channels + ones
XTROWS = 132       # 2 pad + 128 + 2 pad rows in xTe

# d (= key row r - 4j for pair j) -> col offset of its 128-col block in a slab.
# Blocks from even key rows (PE row-tile 0) sit in PSUM bank A (cols 0:512),
# odd-row blocks (tile 1) in bank B (cols 512:896) -- concurrent row-tiled
# matmuls writing the same PSUM bank hang TRN2.
OFF_OF_D = {4: 0, 0: 128, 2: 256, -2: 384, 1: 512, 3: 640, -1: 768}

DELAY_A = 4        # stage-A for pair s-DELAY_A is emitted in iteration s
                   # (2-iter slack so DVE finish-bursts never stall the PE FIFO)


def make_wpos(row_emb, col_emb, mix_emb):
    a = mix_emb.T.astype(np.float64) @ row_emb.astype(np.float64)  # [2,5]
    b = mix_emb.T.astype(np.float64) @ col_emb.astype(np.float64)  # [2,5]
    wp = a[:, :, None] + b[:, None, :]                             # [2,5,5]
    wp = wp - wp.max(axis=0, keepdims=True)
    e = np.exp(wp)
    wp = e / e.sum(axis=0, keepdims=True)
    return wp.reshape(2, 25).astype(np.float32)                    # [m, dh*5+dw]


def make_masks(wpos):
    """wpos-weighted band masks in the trimmed ST layout.

    Returns [128 (kcol), 2 (m), 896] f32; block at OFF_OF_D[d] holds the
    masks for key row r = 4j + d of pair j, cols rho*64 + w."""
    wm = np.zeros((128, 2, SLABW), np.float32)
    for d, base in OFF_OF_D.items():
        for rho in (0, 1):
            dh = d + 2 - 2 * rho
            if not 0 <= dh < 5:
                continue
            for w in range(64):
                for dw in range(5):
                    kc = 2 * w + dw - 2
                    if 0 <= kc < 128:
                        wm[kc, :, base + rho * 64 + w] = wpos[:, dh * 5 + dw]
    return wm


def make_oob():
    """#window entries with out-of-image column, per position in a pair."""
    oob = np.zeros((128, 1), np.float32)
    for rho in (0, 1):
        for w in range(64):
            cnt = sum(1 for dw in range(5) if not 0 <= 2 * w + dw - 2 < 128)
            oob[rho * 64 + w, 0] = 5.0 * cnt
    return oob


def _ap(t, off, dims, p0=0, pn=None):
    a = t[:]
    np_ = pn if pn is not None else a.ap[0][1]
    return bass.AP(tensor=a.tensor, offset=off + p0 * a.ap[0][0],
                   ap=[[a.ap[0][0], np_]] + [list(d) for d in dims])


def build_nc():
    nc = bacc.Bacc("TRN2", target_bir_lowering=False, debug=False, num_devices=NCORES)

    xe_d = nc.dram_tensor("xe", [CIN, 64, IMG], BF16, kind="ExternalInput")
    xo_d = nc.dram_tensor("xo", [CIN, 64, IMG], BF16, kind="ExternalInput")
    xte_d = nc.dram_tensor("xte", [128, XTROWS * XTW], BF16, kind="ExternalInput")
    g2_d = nc.dram_tensor("g2", [CIN, 128], BF16, kind="ExternalInput")
    wv_d = nc.dram_tensor("wve", [XTW, 258], BF16, kind="ExternalInput")
    # wmask + a trailing 2x128-col block whose kc=0 row holds the col-OOB
    # count twice (stage-A adds it to both pairs' den rows in one matmul).
    wm_d = nc.dram_tensor("wmask", [128, 2 * SLABW + 256], BF16, kind="ExternalInput")
    out_d = nc.dram_tensor("out", [HO * HO, OC], BF16, kind="ExternalOutput")

    EXP = mybir.ActivationFunctionType.Exp

    with TileContext(nc) as tc, ExitStack() as ctx:
        sg = ctx.enter_context(tc.tile_pool(name="singles", bufs=1))
        # x: partitions 0:64 even image rows, 64:128 odd rows; 64 rows x 128 cols
        x_sb = sg.tile([128, 64 * IMG], BF16)
        xte_sb = sg.tile([128, XTROWS * XTW], BF16)
        y_sb = sg.tile([128, 4096], BF16)            # queries, dup on both halves
        wm_sb = sg.tile([128, 2 * SLABW + 256], BF16)
        g2_sb = sg.tile([64, 128], BF16)
        wv_sb = sg.tile([XTW, 258], BF16)
        scr_sb = sg.tile([64, 512], BF16)            # HAM warmup scratch

        # sync queue: small critical constants first
        nc.sync.dma_start(out=g2_sb[:], in_=g2_d.ap())
        nc.sync.dma_start(out=wm_sb[:], in_=wm_d.ap())
        nc.sync.dma_start(out=wv_sb[:], in_=wv_d.ap())
        nc.gpsimd.memset(scr_sb[:], 0.0)

        # gpsimd ring (cheap 25ns issue): x chunks only, E/O interleaved.
        # E/O chunk k: 8 packed rows (img rows 16k..16k+15), needed by slab 4k.
        # xte chunks ride the sync ring after the constants (needed later:
        # stage-A runs DELAY_A iterations behind).
        def x_chunk(c8):
            dst_e = _ap(x_sb, c8 * 8 * IMG, [[1, 8 * IMG]], 0, 64)
            dst_o = _ap(x_sb, c8 * 8 * IMG, [[1, 8 * IMG]], 64, 64)
            nc.gpsimd.dma_start(out=dst_e, in_=xe_d.ap()[:, c8 * 8:(c8 + 1) * 8, :])
            nc.gpsimd.dma_start(out=dst_o, in_=xo_d.ap()[:, c8 * 8:(c8 + 1) * 8, :])

        def xte_chunk(c):
            r0 = c * 22
            dst = _ap(xte_sb, r0 * XTW, [[1, 22 * XTW]])
            nc.sync.dma_start(out=dst, in_=xte_d.ap()[:, r0 * XTW:(r0 + 22) * XTW])

        for k in range(8):
            x_chunk(k)
        for c in range(6):
            xte_chunk(c)

        def xrow(r):
            # key row r: [64 partitions (channels), 128 cols] on its parity half
            p = (r & 1) * 64
            return x_sb[p:p + 64, (r >> 1) * IMG:(r >> 1) * IMG + IMG]

        with tc.tile_pool(name="stp", bufs=1, space="PSUM") as stpool, \
             tc.tile_pool(name="ybuf", bufs=1, space="PSUM") as ybuf, \
             tc.tile_pool(name="qp", bufs=1, space="PSUM") as qpool, \
             tc.tile_pool(name="apb", bufs=2, space="PSUM") as apool, \
             tc.tile_pool(name="e2t", bufs=2) as e2t, \
             tc.tile_pool(name="a0p", bufs=5) as a0p, \
             tc.tile_pool(name="qsb", bufs=2) as qsb, \
             tc.tile_pool(name="outsb", bufs=3) as outsb, \
             tc.tile_pool(name="dens", bufs=4) as dens:

            A = {}       # slab -> (a01 group tile, col base)
            QS = {}      # group -> qs tile [65, 512]
            AP2 = {}     # group -> apool tile [128, 258]

            # HAM warmup: ~3.4us of dummy matmuls (no DMA deps) so the PE
            # clock is at 8/8 when the first real matmul issues.
            wup = stpool.tile([128, 2048], F32, name="wup", tag="st")
            for w in range(8):
                nc.tensor.matmul(wup[:, (w % 2) * 512:(w % 2) * 512 + 512],
                                 scr_sb[0:64, 0:128], scr_sb[0:64, 0:512],
                                 start=True, stop=True, skip_group_check=True)

            def make_y(k):
                # y chunk k: queries for strided rows 8k..8k+8 -> y_sb cols
                # k*512..(k+1)*512; needs x even chunk k only.
                yp = ybuf.tile([128, 512], F32)
                rhs = _ap(x_sb, k * 8 * IMG, [[IMG, 8], [2, 64]], 0, 64)
                nc.tensor.matmul(yp[:], g2_sb[:], rhs, start=True, stop=True)
                dst = y_sb[:, k * 512:(k + 1) * 512]
                if k % 2 == 0:
                    nc.scalar.copy(dst, yp[:])
                else:
                    nc.vector.tensor_copy(dst, yp[:])

            def st_mms(s, stp, base):
                n = min(2, NPAIR - s) * 128
                # t=0 (tile0, bank A): pairs s-1, s -> cols 0:256
                if s == 0:
                    nc.tensor.matmul(stp[:, base + 128:base + 256], xrow(0),
                                     y_sb[0:64, 0:128], start=True, stop=True)
                else:
                    nc.tensor.matmul(stp[:, base:base + 256], xrow(4 * s),
                                     y_sb[0:64, (s - 1) * 128:(s + 1) * 128],
                                     start=True, stop=True)
                # t=1 (tile1, bank B): pair s -> cols 512:640
                nc.tensor.matmul(stp[:, base + 512:base + 640], xrow(4 * s + 1),
                                 y_sb[64:128, s * 128:(s + 1) * 128],
                                 start=True, stop=True)
                # t=2 (tile0, bank A): pairs s, s+1 -> cols 256:256+n
                nc.tensor.matmul(stp[:, base + 256:base + 256 + n],
                                 xrow(4 * s + 2),
                                 y_sb[0:64, s * 128:s * 128 + n],
                                 start=True, stop=True)
                # t=3 (tile1, bank B): pairs s, s+1 -> cols 640:640+n
                nc.tensor.matmul(stp[:, base + 640:base + 640 + n],
                                 xrow(4 * s + 3),
                                 y_sb[64:128, s * 128:s * 128 + n],
                                 start=True, stop=True)

            def make_group(G):
                # two slabs 2G, 2G+1: 8 ST matmuls, one exp, one mask op
                stp = stpool.tile([128, 2048], F32, tag="st")
                st_mms(2 * G, stp, 0)
                st_mms(2 * G + 1, stp, 1024)
                e2 = e2t.tile([128, 2 * SLABW], BF16)
                src = bass.AP(tensor=stp[:].tensor, offset=stp[:].offset,
                              ap=[list(stp[:].ap[0]), [1024, 2], [1, SLABW]])
                nc.scalar.activation(out=e2[:], in_=src, func=EXP)
                a01 = a0p.tile([128, 4 * SLABW], BF16)
                e2b = bass.AP(tensor=e2[:].tensor, offset=e2[:].offset,
                              ap=[list(e2[:].ap[0]), [SLABW, 2], [0, 2],
                                  [1, SLABW]])
                wmb = bass.AP(tensor=wm_sb[:].tensor, offset=wm_sb[:].offset,
                              ap=[list(wm_sb[:].ap[0]), [0, 2], [SLABW, 2],
                                  [1, SLABW]])
                nc.vector.tensor_mul(a01[:], e2b, wmb)
                A[2 * G] = (a01, 0)
                A[2 * G + 1] = (a01, 2 * SLABW)
                A.pop(2 * G - 8, None)
                A.pop(2 * G - 7, None)

            def stage_a(g):
                # pairs 2g, 2g+1 -> Q group [65, 512]: m0p0|m0p1|m1p0|m1p1.
                # Q row 64 = den (ones channel; pad rows add sum(wpos) for
                # OOB key rows; the oob matmul adds the col-OOB count).
                qgp = qpool.tile([65, 512], F32)
                pad0 = _ap(xte_sb, 0, [[1, XTW]])
                ds = (0, 1, 2, -2, -1, 3, 4)
                for p in (0, 1):
                    j = 2 * g + p
                    nc.tensor.matmul(qgp[:, p * 128:p * 128 + 128], pad0,
                                     wm_sb[:, 2 * SLABW:2 * SLABW + 128],
                                     start=True, stop=False,
                                     skip_group_check=True)
                    for m in (0, 1):
                        for idx, d in enumerate(ds):
                            r = 4 * j + d
                            lhsT = _ap(xte_sb, (r + 2) * XTW, [[1, XTW]])
                            if r < 0 or r >= IMG:
                                rhs = wm_sb
                                off = OFF_OF_D[d] + m * SLABW
                            else:
                                tile, base = A[r // 4]
                                rhs = tile
                                off = base + OFF_OF_D[d] + m * SLABW
                            nc.tensor.matmul(
                                qgp[:, m * 256 + p * 128:m * 256 + p * 128 + 128],
                                lhsT, rhs[:, off:off + 128],
                                start=(idx == 0 and m == 1),
                                stop=idx == len(ds) - 1,
                                skip_group_check=True)
                qs = qsb.tile([65, 512], BF16)
                nc.scalar.copy(qs[:], qgp[:])
                QS[g] = qs

            def stage_b(g):
                # out[pos,ch'] (+den in col 128) = sum_m wv_m^T Qs_m
                ap2 = apool.tile([128, 258], F32, name="ap2")
                AP2[g] = ap2
                qs = QS.pop(g)
                for p in (0, 1):
                    for m in (0, 1):
                        nc.tensor.matmul(
                            ap2[:, p * 129:p * 129 + 129],
                            qs[0:XTW, m * 256 + p * 128:m * 256 + p * 128 + 128],
                            wv_sb[:, m * 129:(m + 1) * 129],
                            start=m == 0, stop=m == 1,
                            skip_group_check=True)

            def finish_group(g):
                # rec = 1/den (den includes oob via stage-A), out = val*rec
                ap2 = AP2.pop(g)
                rec = dens.tile([128, 2], F32)
                nc.vector.reciprocal(rec[:], _ap(ap2, 128, [[129, 2]]))
                o_sb = outsb.tile([128, 256], BF16)
                rec_b = bass.AP(tensor=rec[:].tensor, offset=rec[:].offset,
                                ap=[list(rec[:].ap[0]), [1, 2], [0, 128]])
                src = bass.AP(tensor=ap2[:].tensor, offset=ap2[:].offset,
                              ap=[list(ap2[:].ap[0]), [129, 2], [1, 128]])
                nc.vector.tensor_mul(o_sb[:], src, rec_b)
                for p in (0, 1):
                    j = 2 * g + p
                    nc.sync.dma_start(out=out_d.ap()[j * 128:(j + 1) * 128, :],
                                      in_=o_sb[:, p * 128:(p + 1) * 128])

            NG = NPAIR // 2
            make_y(0)
            make_y(1)
            for G in range(NG):
                make_group(G)
                if G % 2 == 0 and G // 2 + 2 < 8:
                    make_y(G // 2 + 2)
                if G >= 2:
                    stage_a(G - 2)
                if G >= 3:
                    stage_b(G - 3)
                if G >= 4:
                    finish_group(G - 4)
            for g in range(NG - 2, NG):
                stage_a(g)
                stage_b(g - 1)
                finish_group(g - 2)
            stage_b(NG - 1)
            finish_group(NG - 2)
            finish_group(NG - 1)

    nc.compile()
    return nc


_NC_CACHE = None


def kernel(x, w_q, w_k, w_v, row_emb, col_emb, mix_emb):
    global _NC_CACHE
    x = np.asarray(x, np.float32)
    w_q = np.asarray(w_q, np.float32)
    w_k = np.asarray(w_k, np.float32)
    w_v = np.asarray(w_v, np.float32)
    row_emb = np.asarray(row_emb, np.float32)
    col_emb = np.asarray(col_emb, np.float32)
    mix_emb = np.asarray(mix_emb, np.float32)

    G = (w_q.T @ w_k).astype(ml_dtypes.bfloat16)          # [64, 64]
    g2 = np.hstack([G, G])                                # [64, 128]
    wpos = make_wpos(row_emb, col_emb, mix_emb)
    wmask = np.zeros((128, 2 * SLABW + 256), np.float32)
    wmask[:, 0:2 * SLABW] = make_masks(wpos).reshape(128, 2 * SLABW)
    wmask[0, 2 * SLABW:2 * SLABW + 128] = make_oob()[:, 0]
    wmask[0, 2 * SLABW + 128:] = make_oob()[:, 0]
    wmask = wmask.astype(ml_dtypes.bfloat16)

    # wv_ext: [65, 258]; block m: rows 0:64 col j = w_v[2j+m, c]; row 64 is
    # the den pass-through (-> out col 128).
    wve = np.zeros((XTW, 258), np.float32)
    for m in (0, 1):
        wve[0:64, m * 129:m * 129 + 128] = w_v[m::2, :].T
        wve[64, m * 129 + 128] = 1.0
    wve = wve.astype(ml_dtypes.bfloat16)

    xb = x.astype(ml_dtypes.bfloat16)                      # [B, 64, 128, 128]
    xe = np.ascontiguousarray(xb[:, :, 0::2, :])           # [B, 64, 64, 128]
    xo = np.ascontiguousarray(xb[:, :, 1::2, :])

    # xTe: [B, imgcol 128, row 132, ch 65]; ones channel 64; pad rows 0/1 and
    # 130/131 are zero except the ones channel.
    xte = np.zeros((NCORES, 128, XTROWS, XTW), np.float32)
    xte[:, :, 2:130, 0:64] = x.transpose(0, 3, 2, 1)
    xte[:, :, :, 64] = 1.0
    xte = xte.reshape(NCORES, 128, XTROWS * XTW).astype(ml_dtypes.bfloat16)

    if _NC_CACHE is None:
        _NC_CACHE = build_nc()
    nc = _NC_CACHE

    in_maps = []
    for b in range(NCORES):
        in_maps.append({
            "xe": xe[b],
            "xo": xo[b],
            "xte": xte[b],
            "g2": g2,
            "wve": wve,
            "wmask": wmask,
        })
    res = run_bass_kernel_spmd(nc, in_maps, core_ids=list(range(NCORES)))
    out = np.stack([res.results[b]["out"].astype(np.float32).T.reshape(OC, HO, HO)
                    for b in range(NCORES)])
    return out
